# revision 1
# baseline (speedup 1.0000x reference)
"""Bass/Trainium2 kernel for nn_LocalLayer_9603546874456 (GCN message passing).

Math: out = leaky_relu(x @ W + b) for all B*N nodes, except the first N
flattened rows (batch 0), which aggregate neighbors:
    out[:N] = leaky_relu(M @ (x[:N] @ W) + b),  M = norm_adj.T + diag(1/deg)
Since M @ (x0 @ W) == (M @ x0) @ W, we fold the tiny 62x62 aggregation into a
host-side premultiply of x's first 62 rows, making the device kernel a uniform
memory-bound fused matmul + bias + leaky_relu.

Device strategy (per core, data-parallel over batch):
  - Host pre-transposes each shard to FIN-major xt (128, R_CORE) so the
    contraction dim (FIN=128) lands on SBUF partitions with contiguous DMA.
  - W (128, 64) is the stationary matmul operand; x streams as the moving
    operand in N=512 chunks.
  - Two row-chunks are packed into the 128 PSUM/SBUF partitions (features
    0-63 of chunk 2i on partitions 0-63, of chunk 2i+1 on partitions 64-127)
    so stores run at full 128-partition DMA bandwidth.
"""

import os
import sys

import numpy as np

B, N, FIN, FOUT = 8192, 62, 128, 64
R_TOTAL = B * N  # 507904
N_CORES = 8
R_CORE = R_TOTAL // N_CORES  # 63488
F_PAIR = 2048  # x columns consumed per iteration (two 1024-row chunks)
F_HALF = F_PAIR // 2  # 1024
MM_N = 512  # moving free dim per matmul (fp32 max)
LEAKY_SLOPE = 0.01

try:
    import concourse  # noqa: F401
except ImportError:  # pragma: no cover
    sys.path.insert(0, "/opt/trn_rl_repo")


def build_program(r_core: int = R_CORE, act_mode: str = "lrelu"):
    """Build + compile the SPMD Bass program (same program for all cores).

    act_mode: 'lrelu' uses the single-op ScalarE Lrelu LUT;
              'fallback' uses Identity+bias (ACT) then max(z, 0.01*z) (DVE),
              which the python CoreSim can execute.
    """
    import concourse.bacc as bacc
    import concourse.tile as tile
    from concourse import mybir

    assert r_core % F_PAIR == 0
    n_iter = r_core // F_PAIR
    yt_cols = r_core // 2

    nc = bacc.Bacc(
        "TRN2",
        target_bir_lowering=False,
        debug=False,
        num_devices=N_CORES,
    )
    xt = nc.dram_tensor("xt", [FIN, r_core], mybir.dt.float32, kind="ExternalInput").ap()
    w = nc.dram_tensor("w", [FIN, FOUT], mybir.dt.float32, kind="ExternalInput").ap()
    b2 = nc.dram_tensor("b2", [128, 1], mybir.dt.float32, kind="ExternalInput").ap()
    yt = nc.dram_tensor(
        "yt", [128, yt_cols], mybir.dt.float32, kind="ExternalOutput"
    ).ap()

    with tile.TileContext(nc) as tc:
        with (
            tc.tile_pool(name="const", bufs=1) as cpool,
            tc.tile_pool(name="xin", bufs=3) as xpool,
            tc.tile_pool(name="yout", bufs=3) as ypool,
            tc.tile_pool(name="ps", bufs=2, space="PSUM") as pspool,
        ):
            w_sb = cpool.tile([FIN, FOUT], mybir.dt.float32)
            nc.sync.dma_start(w_sb[:], w[:])
            b_sb = cpool.tile([128, 1], mybir.dt.float32)
            nc.sync.dma_start(b_sb[:], b2[:])

            for i in range(n_iter):
                xtile = xpool.tile([128, F_PAIR], mybir.dt.float32)
                nc.sync.dma_start(xtile[:], xt[:, i * F_PAIR : (i + 1) * F_PAIR])

                ps = pspool.tile([128, F_HALF], mybir.dt.float32)
                for j in range(F_HALF // MM_N):
                    sl = slice(j * MM_N, (j + 1) * MM_N)
                    # chunk 2i -> psum partitions 0:64
                    nc.tensor.matmul(
                        ps[0:FOUT, sl], w_sb[:], xtile[:, sl], start=True, stop=True
                    )
                    # chunk 2i+1 -> psum partitions 64:128
                    nc.tensor.matmul(
                        ps[FOUT:128, sl],
                        w_sb[:],
                        xtile[:, F_HALF + j * MM_N : F_HALF + (j + 1) * MM_N],
                        start=True,
                        stop=True,
                    )

                otile = ypool.tile([128, F_HALF], mybir.dt.float32)
                if act_mode == "lrelu":
                    nc.scalar.activation(
                        otile[:],
                        ps[:],
                        mybir.ActivationFunctionType.Lrelu,
                        bias=b_sb[:],
                        scale=1.0,
                        alpha=LEAKY_SLOPE,
                    )
                else:
                    ztile = ypool.tile([128, F_HALF], mybir.dt.float32, tag="z")
                    nc.scalar.activation(
                        ztile[:],
                        ps[:],
                        mybir.ActivationFunctionType.Identity,
                        bias=b_sb[:],
                        scale=1.0,
                    )
                    # leaky = max(z, slope * z)
                    nc.vector.scalar_tensor_tensor(
                        otile[:],
                        ztile[:],
                        LEAKY_SLOPE,
                        ztile[:],
                        op0=mybir.AluOpType.mult,
                        op1=mybir.AluOpType.max,
                    )
                nc.sync.dma_start(yt[:, i * F_HALF : (i + 1) * F_HALF], otile[:])

    nc.compile()
    return nc


def _aggregation_matrix(adj: np.ndarray) -> np.ndarray:
    """M such that reference's first-block output = (M @ x0) @ W + b."""
    adj = adj.astype(np.float32)
    deg = 1.0 + adj.sum(axis=0)  # incoming degree + self loop
    d = deg.astype(np.float32) ** -0.5
    norm_adj = adj * d[:, None] * d[None, :]
    return norm_adj.T + np.diag((d * d).astype(np.float32))


def prepare_inputs(x, adj, W, b):
    """Shard + reformat host-side. Returns in_maps for run_bass_kernel_spmd."""
    x_flat = np.ascontiguousarray(x.reshape(-1, FIN), dtype=np.float32)
    M = _aggregation_matrix(adj)
    W = np.ascontiguousarray(W, dtype=np.float32)
    b = np.asarray(b, dtype=np.float32)
    b2 = np.concatenate([b, b]).reshape(128, 1).astype(np.float32)

    in_maps = []
    for c in range(N_CORES):
        shard = x_flat[c * R_CORE : (c + 1) * R_CORE]
        if c == 0:
            shard = shard.copy()
            shard[:N] = (M @ shard[:N]).astype(np.float32)
        xt_c = np.ascontiguousarray(shard.T)  # (128, R_CORE)
        in_maps.append({"xt": xt_c, "w": W, "b2": b2})
    return in_maps


def unpack_outputs(results) -> np.ndarray:
    """results: list of per-core dicts with 'yt' (128, R_CORE//2)."""
    y_parts = []
    for c in range(N_CORES):
        yt_c = np.asarray(results[c]["yt"])  # (128, R_CORE//2)
        n_iter = R_CORE // F_PAIR
        # [h, f, i, c] -> row = i*F_PAIR + h*F_HALF + c
        yt3 = yt_c.reshape(2, FOUT, n_iter, F_HALF)
        y_c = yt3.transpose(2, 0, 3, 1).reshape(R_CORE, FOUT)
        y_parts.append(y_c)
    y = np.concatenate(y_parts, axis=0)
    return y.reshape(B, N, FOUT)


_PROGRAM_CACHE = {}


def _get_program(act_mode: str = "lrelu"):
    key = (R_CORE, act_mode)
    if key not in _PROGRAM_CACHE:
        _PROGRAM_CACHE[key] = build_program(R_CORE, act_mode)
    return _PROGRAM_CACHE[key]


def kernel(x, adj, W, b, _act_mode: str = "lrelu", _trace: bool = False):
    from concourse.bass_utils import run_bass_kernel_spmd

    nc = _get_program(_act_mode)
    in_maps = prepare_inputs(x, adj, W, b)
    res = run_bass_kernel_spmd(nc, in_maps, list(range(N_CORES)), trace=_trace)
    out = unpack_outputs(res.results)
    if _trace:
        kernel.last_exec_time_ns = res.exec_time_ns
        kernel.last_results = res
    return out
